# revision 32
# baseline (speedup 1.0000x reference)
"""Tensor-parallel causal attention (GQA, rotary) for Trainium2, 8 NeuronCores.

Problem: x[2,2048,2048] -> QKV proj -> rotary -> 32-head causal attention
(8 kv heads, head_dim 64) -> out @ wo, fp32 reference.

Sharding (tensor-parallel over heads): core c owns q heads [4c,4c+4) and kv
head c. Each core computes its heads' attention and a partial output
projection partial_c = attn_c @ wo[256c:256c+256]; the host sums 8 partials.

bf16 operands end-to-end (fp32 psum accumulation), engines rebalanced so
ACT does only exp, DVE does the psum-touching work, Pool SBUF-only work:
  A) QKV projection (bf16): stationary weights (first k-chunk DMA'd
     separately so matmuls start ~2.5us in), xT streamed on two HWDGE
     queues, m-outer accumulation over a 4-bank psum ring. Rotary: DVE
     does the full-tile psum*cos/sin muls and the [32,512] pair-tile
     combines into QF (bf16, 32-aligned partition-shifted writes; all on
     DVE -- Pool stores feeding PE raced intermittently on HW); K via
     base-0 [32,512] tiles (walrus requires equal SB input bases). V.T
     built by XBAR dma_start_transpose into a 16-aligned-stride VH (no PE
     transpose); softmax-denominator ones-columns memset once.
  C) Scores transposed (S.T = K.T-tile.T @ Q.T), 2 heads row-packed,
     sc double-buffered. exp on ACT per head-half over the causally-live
     column ranges only; diagonal tiles masked POST-exp by a DVE multiply
     with a [128,128] 0/1 triangle const (plain 2D slices only -- 3D band
     APs and Pool ops in this path were racy on HW). PV (bf16)
     accumulates (V|1).T @ P.T; row 64 =
     denominator: DVE reciprocal -> ones-matmul broadcast -> DVE
     normalize into ATT.
  D) Output projection bf16, emitted one (b,j) group behind attention so
     its matmuls fill the exp-bound pipeline gaps; 2-matmul psum tiles
     [128,512] (bufs=2), DVE evicts to bf16, OUT DMAs on the sync queue
     only (scalar-queue DMAs would block exp dispatch in the ACT
     sequencer), OUT partials written bf16 (host sums in fp32).

PSUM budget: A: qkv 4; C/D: sc 2x2 + pvA 1 + pvB 1 + pd/bc 2 = 8 banks.
"""
import numpy as np

B, S, D = 2, 2048, 2048
H, KV, HD = 32, 8, 64
NCORES = 8
HPC = H // NCORES          # 4 q heads per core
TOKS = B * S               # 4096
DCH = D // 128             # 16 contraction chunks
NBLK = TOKS // 512         # 8 token blocks of 512
QB = 512                   # q block size (phase C)
KTILES = S // 128          # 16 k tiles per batch

_CACHE = {}


def _build(reps=1, debug=False, phases="acd"):
    """reps>1 statically unrolls the whole pipeline for timing runs
    (dispatch overhead cancels in the difference between reps values)."""
    import concourse.bacc as bacc
    import concourse.mybir as mybir
    from concourse import tile

    F32 = mybir.dt.float32
    F32R = mybir.dt.float32r
    BF16 = mybir.dt.bfloat16
    EXP = mybir.ActivationFunctionType.Exp

    nc = bacc.Bacc()
    xT = nc.declare_dram_parameter("xT", [D, TOKS], BF16, isOutput=False)
    W = nc.declare_dram_parameter("W", [D, 384], BF16, isOutput=False)
    WO = nc.declare_dram_parameter("WO", [256, D], BF16, isOutput=False)
    CS = nc.declare_dram_parameter("CS", [128, S], F32, isOutput=False)
    SN = nc.declare_dram_parameter("SN", [128, S], F32, isOutput=False)
    OUT = nc.declare_dram_parameter("OUT", [TOKS, D], BF16, isOutput=True)
    if debug:
        QF_d = nc.declare_dram_parameter("QF_d", [128, 2 * TOKS], BF16, isOutput=True)
        KF_d = nc.declare_dram_parameter("KF_d", [128, TOKS], BF16, isOutput=True)
        VH_d = nc.declare_dram_parameter("VH_d", [128, 2 * KTILES * 80], BF16, isOutput=True)
        ATT_d = nc.declare_dram_parameter("ATT_d", [128, 2 * TOKS], BF16, isOutput=True)

    with tile.TileContext(nc) as tc:
        with tc.tile_pool(name="const", bufs=1) as cp:
            W_sb = cp.tile([128, DCH * 384], BF16)       # 12KB/part
            Wr = W.rearrange("(k p) c -> p k c", p=128)
            W3 = W_sb[:].rearrange("p (k c) -> p k c", k=DCH)
            nc.sync.dma_start(out=W3[:, 0:1], in_=Wr[:, 0:1])
            nc.sync.dma_start(out=W3[:, 1:DCH], in_=Wr[:, 1:DCH])
            CS_sb = cp.tile([128, S], F32)
            SN_sb = cp.tile([128, S], F32)
            WO_sb = cp.tile([128, 2 * D], BF16)          # 8KB/part; loaded late
            QF = cp.tile([128, 2 * TOKS], BF16)          # pairtile p at cols p*TOKS
            KF = cp.tile([128, TOKS], BF16)              # rows 0:64 K, 64:128 replica
            VH = cp.tile([128, 2 * KTILES * 80], BF16)   # (b*16+t)*80 | V.T,1,pad
            # stride 80 (16-aligned) so XBAR dma-transpose dest offsets are legal
            ATT = cp.tile([128, 2 * TOKS], BF16)         # ftile f at cols f*TOKS
            # ones columns of VH (softmax denominator trick), written once
            vh3 = VH[:].rearrange("p (i x) -> p i x", i=2 * KTILES)
            nc.vector.memset(vh3[:, :, 64:65], 1.0)
            ones_row = cp.tile([1, 64], F32R)
            onef = cp.tile([1, 64], F32)
            nc.vector.memset(onef[:], 1.0)
            nc.vector.tensor_copy(ones_row[:], onef[:])
            # 0/1 lower-triangle-live const: TRI[k, q] = 1 if q >= k else 0
            TRI = cp.tile([128, 128], BF16)
            nc.vector.memset(TRI[:], 1.0)
            nc.gpsimd.affine_select(
                out=TRI[:], in_=TRI[:],
                compare_op=mybir.AluOpType.is_ge,
                fill=0.0, base=0, pattern=[[1, 128]],
                channel_multiplier=-1)

            def _emit_body():
              # ---------------- Phase A: QKV projection + rotary + V transpose
              with (
                  tc.tile_pool(name="xa", bufs=5) as xap,
                  tc.tile_pool(name="pa", bufs=1, space="PSUM") as pap,
                  tc.tile_pool(name="ta", bufs=2) as tap,
              ):
                  for n in range(NBLK):
                      b = n // 4
                      ccols = slice((n % 4) * 512, (n % 4) * 512 + 512)
                      ncols = slice(n * 512, (n + 1) * 512)
                      pss = [pap.tile([128, 512], F32, tag="qkv", bufs=6,
                                      name=f"ps_{n}_{m}") for m in range(3)]
                      xr = xT.rearrange("(k p) t -> p k t", p=128)
                      qtr = DCH // 4
                      xhs = []
                      for hh in range(4):
                          xh = xap.tile([128, qtr * 512], BF16, tag="xt",
                                        name=f"xt_{n}_{hh}")
                          xeng = nc.sync if hh % 2 == 0 else nc.scalar
                          xeng.dma_start(out=xh[:],
                                         in_=xr[:, hh * qtr:(hh + 1) * qtr, ncols])
                          xhs.append(xh)
                      # m-outer: with bufs=4 the next block's m=0 chain can
                      # start while this block's m=2 psum is still being read
                      for m in range(3):
                          for k in range(DCH):
                              xt = xhs[k // qtr][:, (k % qtr) * 512:(k % qtr + 1) * 512]
                              nc.tensor.matmul(
                                  pss[m][:],
                                  W_sb[:, k * 384 + m * 128: k * 384 + (m + 1) * 128],
                                  xt, start=(k == 0), stop=(k == DCH - 1))
                      if n == 0:
                          # rotary tables needed only ~10us in; don't let them
                          # delay the x/W stream at kernel start
                          nc.scalar.dma_start(out=CS_sb[:], in_=CS[:])
                          nc.scalar.dma_start(out=SN_sb[:], in_=SN[:])
                      # rotary Q: psum0 = evens of 4 heads, psum1 = odds (DVE muls)
                      t1 = tap.tile([128, 512], F32, tag="t1", name=f"t1_{n}")
                      t2 = tap.tile([128, 512], F32, tag="t2", name=f"t2_{n}")
                      t3 = tap.tile([128, 512], F32, tag="t3", name=f"t3_{n}")
                      t4 = tap.tile([128, 512], F32, tag="t4", name=f"t4_{n}")
                      nc.vector.tensor_mul(t1[:], pss[0][:], CS_sb[:, ccols])
                      nc.vector.tensor_mul(t2[:], pss[1][:], SN_sb[:, ccols])
                      nc.vector.tensor_mul(t3[:], pss[0][:], SN_sb[:, ccols])
                      nc.vector.tensor_mul(t4[:], pss[1][:], CS_sb[:, ccols])
                      # pair-tile combines: Pool does 6 (SBUF-only), DVE 2
                      # (engine balance: Pool add ~1.0us/op vs DVE ~0.53)
                      for h in range(HPC):
                          base = (h // 2) * TOKS + n * 512
                          ee = nc.vector  # Pool stores feeding PE raced on HW
                          ee.tensor_sub(
                              QF[64 * (h % 2): 64 * (h % 2) + 32, base: base + 512],
                              t1[32 * h:32 * h + 32, :], t2[32 * h:32 * h + 32, :])
                          ee.tensor_add(
                              QF[64 * (h % 2) + 32: 64 * (h % 2) + 64, base: base + 512],
                              t3[32 * h:32 * h + 32, :], t4[32 * h:32 * h + 32, :])
                      # rotary K: rows 0:32 even, 32:64 odd of pss[2].
                      # NOTE: SB+SB tensor ops need equal input base partitions
                      # (walrus verifier), so the muls land in base-0 tiles
                      tk1 = tap.tile([32, 512], F32, tag="t1", name=f"tk1_{n}")
                      tk2 = tap.tile([32, 512], F32, tag="t2", name=f"tk2_{n}")
                      nc.vector.tensor_mul(tk1[:], pss[2][0:32, :], CS_sb[0:32, ccols])
                      nc.vector.tensor_mul(tk2[:], pss[2][32:64, :], SN_sb[32:64, ccols])
                      nc.vector.tensor_sub(KF[0:32, ncols], tk1[:], tk2[:])
                      tk3 = tap.tile([32, 512], F32, tag="t3", name=f"tk3_{n}")
                      tk4 = tap.tile([32, 512], F32, tag="t4", name=f"tk4_{n}")
                      nc.vector.tensor_mul(tk3[:], pss[2][0:32, :], SN_sb[0:32, ccols])
                      nc.vector.tensor_mul(tk4[:], pss[2][32:64, :], CS_sb[32:64, ccols])
                      nc.vector.tensor_add(KF[32:64, ncols], tk3[:], tk4[:])
                      # replicate this block's K rows for 2-head row packing
                      nc.scalar.dma_start(out=KF[64:128, ncols], in_=KF[0:64, ncols])
                      # V: evict rows 64:128 to bf16, XBAR-transpose into VH
                      vs = tap.tile([64, 512], BF16, tag="vs", name=f"vs_{n}")
                      nc.scalar.copy(vs[:], pss[2][64:128, :])
                      for q in range(4):
                          t_global = (n % 4) * 4 + q      # ktile within batch
                          idx = b * KTILES + t_global
                          teng = nc.sync if q % 2 == 0 else nc.scalar
                          teng.dma_start_transpose(
                              out=VH[:, idx * 80: idx * 80 + 64],
                              in_=vs[:, q * 128:(q + 1) * 128])

              # WO needed only from phase D; late emission = low priority
              for f in range(2):
                  nc.sync.dma_start(out=WO_sb[:, f * D:(f + 1) * D],
                                    in_=WO[f * 128:(f + 1) * 128, :])

              if "c" not in phases:
                  return
              # ------- Phase C+D fused: attention then projection per (b, j)
              with (
                  tc.tile_pool(name="sc", bufs=2, space="PSUM") as scp,
                  tc.tile_pool(name="pv", bufs=1, space="PSUM") as pvp,
                  tc.tile_pool(name="pd", bufs=1, space="PSUM") as pdp,
                  tc.tile_pool(name="pt", bufs=6) as ptp,
                  tc.tile_pool(name="nm", bufs=2) as nmp,
                  tc.tile_pool(name="od", bufs=4) as odp,
              ):
                  def _emit_proj(b, j):
                      # output projection for (b, j)'s 4 token tiles
                      for mq in range(4):
                          mt = b * 16 + j * 4 + mq
                          for hf in range(2):
                              os_ = odp.tile([128, 1024], BF16, tag="od",
                                             name=f"od_{mt}_{hf}")
                              for nb in range(2):
                                  ps = pdp.tile([128, 512], F32, tag="pd",
                                                bufs=2, name=f"pd_{mt}_{hf}_{nb}")
                                  col = hf * 1024 + nb * 512
                                  for f in range(2):
                                      nc.tensor.matmul(
                                          ps[:],
                                          ATT[:, f * TOKS + mt * 128:
                                              f * TOKS + (mt + 1) * 128],
                                          WO_sb[:, f * D + col: f * D + col + 512],
                                          start=(f == 0), stop=(f == 1))
                                  if (mt + hf + nb) % 2 == 0:
                                      nc.vector.tensor_copy(
                                          os_[:, nb * 512:(nb + 1) * 512], ps[:])
                                  else:
                                      nc.scalar.copy(
                                          os_[:, nb * 512:(nb + 1) * 512], ps[:])
                              # sync queue only: a dma_start on nc.scalar sits
                              # in the ACT sequencer and its sem-wait blocks
                              # the exp dispatches queued behind it
                              nc.sync.dma_start(
                                  out=OUT[mt * 128:(mt + 1) * 128,
                                          hf * 1024:(hf + 1) * 1024],
                                  in_=os_[:])

                  pending_proj = None
                  for b in range(B):
                      for j in range(4):          # q block of 512 within batch
                          for pr in range(2):     # head pair
                              qc = slice(b * S + j * 512, b * S + (j + 1) * 512)
                              pvA = pvp.tile([65, 512], F32, tag="pvA", name=f"pvA_{b}_{pr}_{j}")
                              pvB = pvp.tile([65, 512], F32, tag="pvB", name=f"pvB_{b}_{pr}_{j}")
                              nk = 4 * (j + 1)
                              for t in range(nk):
                                  kc = slice(b * S + t * 128, b * S + (t + 1) * 128)
                                  sc = scp.tile([128, 1024], F32, tag="sc",
                                                name=f"sc_{b}_{pr}_{j}_{t}")
                                  # q < 128*i of a diagonal tile is fully masked;
                                  # don't compute those score columns
                                  qskip = max(0, (t - 4 * j) * 128)
                                  q0 = pr * TOKS + b * S + j * 512
                                  nc.tensor.matmul(
                                      sc[:, qskip:512], KF[0:64, kc],
                                      QF[0:64, q0 + qskip: q0 + 512],
                                      start=True, stop=True)
                                  nc.tensor.matmul(
                                      sc[:, 512 + qskip:1024], KF[64:128, kc],
                                      QF[64:128, q0 + qskip: q0 + 512],
                                      start=True, stop=True)
                                  pt = ptp.tile([128, 1024], BF16, tag="pt",
                                                name=f"pt_{b}_{pr}_{j}_{t}")
                                  # exp per head-half as plain 2D slices (3D
                                  # band APs here raced with PV on HW: the
                                  # second half's dep was lost intermittently)
                                  nc.scalar.activation(pt[:, qskip:512],
                                                       sc[:, qskip:512],
                                                       EXP, scale=0.125)
                                  nc.scalar.activation(pt[:, 512 + qskip:1024],
                                                       sc[:, 512 + qskip:1024],
                                                       EXP, scale=0.125)
                                  i = t - 4 * j
                                  if i >= 0:
                                      # diagonal: zero the masked triangle in the
                                      # 128-wide band of each half post-exp
                                      for half in range(2):
                                          b0c = half * 512 + qskip
                                          nc.vector.tensor_mul(
                                              pt[:, b0c:b0c + 128],
                                              pt[:, b0c:b0c + 128], TRI[:])
                                  # masked P columns are zero; skip them in PV too
                                  vcol = VH[:, (b * KTILES + t) * 80:
                                            (b * KTILES + t) * 80 + 65]
                                  nc.tensor.matmul(pvA[:, qskip:512], vcol,
                                                   pt[:, qskip:512],
                                                   start=(t == 0), stop=(t == nk - 1))
                                  nc.tensor.matmul(pvB[:, qskip:512], vcol,
                                                   pt[:, 512 + qskip:1024],
                                                   start=(t == 0), stop=(t == nk - 1))
                              # normalize: reciprocal of denominator row, SBUF
                              # broadcast DMA across 64 partitions, multiply
                              for hh, pv in ((0, pvA), (1, pvB)):
                                  h = 2 * pr + hh
                                  r_sb = nmp.tile([1, 512], F32R, tag="r",
                                                  name=f"r_{b}_{pr}_{j}_{hh}")
                                  with nc.allow_low_precision("fp32r recip feeds normalize"):
                                      nc.vector.reciprocal(r_sb[:], pv[64:65, :])
                                  bc = pdp.tile([64, 512], F32, tag="pd", bufs=2,
                                                name=f"bc_{b}_{pr}_{j}_{hh}")
                                  nc.tensor.matmul(bc[:], ones_row[:], r_sb[:],
                                                   start=True, stop=True)
                                  rb = nmp.tile([64, 512], F32, tag="rb",
                                                name=f"rb_{b}_{pr}_{j}_{hh}")
                                  nc.vector.tensor_copy(rb[:], bc[:])
                                  dst = ATT[64 * (h % 2): 64 * (h % 2) + 64,
                                            (h // 2) * TOKS + b * S + j * 512:
                                            (h // 2) * TOKS + b * S + (j + 1) * 512]
                                  nc.vector.tensor_mul(dst, pv[0:64, :], rb[:])

                          # emit the PREVIOUS group's projection here: its
                          # priority lands between this group's attention and
                          # the next one's, so proj matmuls fill ACT-bound gaps
                          if "d" in phases:
                              if pending_proj is not None:
                                  _emit_proj(*pending_proj)
                              pending_proj = (b, j)
                  if "d" in phases and pending_proj is not None:
                      _emit_proj(*pending_proj)

            for _ in range(reps):
                _emit_body()
            if debug:
                nc.sync.dma_start(out=QF_d[:], in_=QF[:])
                nc.sync.dma_start(out=KF_d[:], in_=KF[:])
                nc.sync.dma_start(out=VH_d[:], in_=VH[:])
                nc.sync.dma_start(out=ATT_d[:], in_=ATT[:])

    nc.compile()
    return nc


def _prep_inputs(x, freqs_cos, freqs_sin, wq, wk, wv, wo):
    """Host-side shard prep. Returns per-core input dicts."""
    import ml_dtypes
    bf16 = ml_dtypes.bfloat16
    x = np.asarray(x, dtype=np.float32)
    fc = np.asarray(freqs_cos, dtype=np.float32)
    fs = np.asarray(freqs_sin, dtype=np.float32)
    wq = np.asarray(wq, dtype=np.float32)
    wk = np.asarray(wk, dtype=np.float32)
    wv = np.asarray(wv, dtype=np.float32)
    wo = np.asarray(wo, dtype=np.float32)

    xT = np.ascontiguousarray(x.transpose(2, 0, 1).reshape(D, TOKS)).astype(bf16)
    CSa = np.ascontiguousarray(np.tile(fc.T, (4, 1)))   # [128, S]
    SNa = np.ascontiguousarray(np.tile(fs.T, (4, 1)))

    in_maps = []
    for c in range(NCORES):
        cols = np.empty(384, dtype=np.int64)
        for h in range(HPC):
            for p in range(32):
                cols[32 * h + p] = (HPC * c + h) * HD + 2 * p          # Q even
                cols[128 + 32 * h + p] = (HPC * c + h) * HD + 2 * p + 1  # Q odd
        qW = wq[:, cols[:256]]
        kcols = np.empty(64, dtype=np.int64)
        kcols[:32] = HD * c + 2 * np.arange(32)
        kcols[32:] = HD * c + 2 * np.arange(32) + 1
        kW = wk[:, kcols]
        vW = wv[:, HD * c: HD * (c + 1)]
        Wc = np.ascontiguousarray(np.concatenate([qW, kW, vW], axis=1)).astype(bf16)
        WOc = np.ascontiguousarray(wo[256 * c: 256 * (c + 1), :]).astype(bf16)
        in_maps.append({"xT": xT, "W": Wc, "WO": WOc, "CS": CSa, "SN": SNa})
    return in_maps


def kernel(x, freqs_cos, freqs_sin, wq, wk, wv, wo):
    from concourse.bass_utils import run_bass_kernel_spmd

    if "nc" not in _CACHE:
        _CACHE["nc"] = _build()
    nc = _CACHE["nc"]
    in_maps = _prep_inputs(x, freqs_cos, freqs_sin, wq, wk, wv, wo)
    res = run_bass_kernel_spmd(nc, in_maps, list(range(NCORES)))
    out = np.zeros((TOKS, D), dtype=np.float32)
    for c in range(NCORES):
        out += res.results[c]["OUT"].astype(np.float32)
    return out.reshape(B, S, D)
